# revision 26
# baseline (speedup 1.0000x reference)
"""LocalAttention2d Bass kernel for 8 Trainium2 NeuronCores.

Strategy: pure data parallel over batch (8 batches/core).  The module only
attends over an 8x8 window of data-dependent spatial positions per batch.
All valid window columns are literally p1+offs (clipping only produces
invalid, masked positions), so each (batch, window-row, col-half) is ONE
contiguous 4-row block of the spatial table.  The kernel computes 128 block
indices on-device and fetches all 512 needed rows with a SINGLE indirect
DMA of 128 x 2KB descriptors from a host-padded [B*(H*W+16), D] table (the
+-8 row pads absorb edge blocks; those positions are invalid and masked).

Layout: partition p = b*16 + i*2 + jg (batch, window-row, col-half); the
index math runs on tiny [16, 8] r/c tiles, is expanded to [128, 1] block
indices by one selector matmul, and the Gaussian shift + validity mask are
folded into the scores PSUM with per-batch-constant and banded matmuls
using the linear mask -1024 + 512*(vr+vc) (exact in f32 at this scale), so
exp() reads PSUM directly.  Softmax is unnormalized: the weighted sum and
the denominator are both matmuls against exp scores, normalized at the end
by a per-partition reciprocal.  Score and weighted-sum matmuls run in
fp32r (single PE pass).

Host-side work is limited to data-INdependent layout prep (transposes of
q / c_t / W_p, constant tables, zero padding); every data-dependent step
(p_t, rounding, window indices, shift, softmax, output) runs on-device.
"""

import sys

import numpy as np

try:
    import concourse.bass_utils as _bu
except ImportError:  # fresh grading dir: fall back to the repo checkout
    sys.path.insert(0, "/opt/trn_rl_repo")
    import concourse.bass_utils as _bu

import concourse.bacc as bacc
import concourse.bass as bass
import concourse.mybir as mybir
import concourse.tile as tile
from concourse.bass import IndirectOffsetOnAxis

B, D, H, W = 64, 128, 128, 128
CSZ = 256
R = 8                     # window rows == cols
NCORES = 8
BPC = B // NCORES         # batches per core
HW = H * W
PADB = 132                # zero rows before each batch (absorbs rr_r=0 blocks)
PADE = 8                  # zero rows after each batch
BSTR = HW + PADB + PADE   # padded batch stride (rows)
ROWS = H + 1              # 129, padded row count in the reference
NS = 4                    # strips (col within 4-row block)
F32 = mybir.dt.float32
F32R = mybir.dt.float32r
I32 = mybir.dt.int32

AOP = mybir.AluOpType
ACT = mybir.ActivationFunctionType
AXL = mybir.AxisListType

BIGC = 1024.0             # mask constants: exact cancellation at 2^10 scale
HALFC = 512.0
FAKEC = -2048.0

# auxS [128, 174] (critical-path constants, first DMA):
#   0:8 ct0 | 8:16 ct1 | 16:18 wp0 | 18:20 wp1 | 20:36 selc16 (parts 0:2)
#   | 36:37 oi16 (parts 0:16) | 37:165 E12c (parts 0:16)
#   | 165:173 selmask | 173:174 jgboff
AUXS_W = 174
# auxL [128, 1064]: 0:128 wa0 | 128:256 wa1 | 256:384 ident
#   | 384:896 B_s x4 (parts 0:16) | 896:1024 bsel (parts 0:8)
#   | 1024:1056 constm8 (parts 0:8) | 1056:1064 fold8 (parts 0:32)
AUXL_W = 1064


def _build():
    nc = bacc.Bacc(
        "TRN2",
        target_bir_lowering=False,
        debug=False,
        num_devices=NCORES,
    )

    qtab = nc.dram_tensor("qtab", [BPC * BSTR, D], F32, kind="ExternalInput")
    auxS = nc.dram_tensor("auxS", [128, AUXS_W], F32, kind="ExternalInput")
    auxL = nc.dram_tensor("auxL", [128, AUXL_W], F32, kind="ExternalInput")
    out = nc.dram_tensor("out", [BPC, D], F32, kind="ExternalOutput")

    with tile.TileContext(nc) as tc:
        with (
            tc.tile_pool(name="sb", bufs=1) as sp,
            tc.tile_pool(name="ps", bufs=1, space="PSUM") as pp,
        ):
            # ---- input DMAs: small critical constants first ---------------
            aS = sp.tile([128, AUXS_W], F32)
            nc.sync.dma_start(out=aS[:], in_=auxS[:])
            aL = sp.tile([128, AUXL_W], F32)
            nc.sync.dma_start(out=aL[:], in_=auxL[:])

            ct0 = aS[:, 0:8]
            ct1 = aS[:, 8:16]
            wp0 = aS[:, 16:18]
            wp1 = aS[:, 18:20]
            selc16 = aS[0:2, 20:36]
            oi16 = aS[0:16, 36:37]
            E12c = aS[0:16, 37:165]
            selmask = aS[:, 165:173]
            jgboff = aS[:, 173:174]
            wa0 = aL[:, 0:128]
            wa1 = aL[:, 128:256]
            ident = aL[:, 256:384]
            bsel = aL[0:8, 896:1024]
            constm8 = aL[0:8, 1024:1056]
            fold8 = aL[0:32, 1056:1064]

            ones2 = sp.tile([128, 2], F32R)
            nc.gpsimd.memset(ones2[:].bitcast(F32), 1.0)

            # ---- p_t: ptT[2,8] = (W_p c_t)^T, spread 128*sigmoid to [16,8] -
            ptcomb = pp.tile([16, 2 * BPC], F32)
            ptT_ps = ptcomb[0:2, BPC : 2 * BPC]
            nc.tensor.matmul(out=ptT_ps, lhsT=wp0, rhs=ct0, start=True, stop=False, skip_group_check=True)
            nc.tensor.matmul(out=ptT_ps, lhsT=wp1, rhs=ct1, start=False, stop=True, skip_group_check=True)
            sig8 = sp.tile([2, BPC], F32)
            nc.scalar.activation(out=sig8[:], in_=ptT_ps, func=ACT.Sigmoid)
            # rows 0:8 = r-part (p_t[:,0]), rows 8:16 = c-part (p_t[:,1])
            ptb_ps = ptcomb[0:16, 0:BPC]
            nc.tensor.matmul(out=ptb_ps, lhsT=selc16, rhs=sig8[:], start=True, stop=True, skip_group_check=True)

            # ---- round via the +-2^23 trick (one op) ----------------------
            prf = sp.tile([16, BPC], F32)
            nc.vector.tensor_scalar(
                out=prf[:], in0=ptb_ps, scalar1=8388608.0, scalar2=8388608.0,
                op0=AOP.add, op1=AOP.subtract,
            )

            # ---- window values: a=max(p+o,0); rr=a*(a<129); rm1=max(rr-1,0)
            aa = sp.tile([16, BPC], F32)
            nc.vector.tensor_scalar(
                out=aa[:], in0=prf[:], scalar1=oi16, scalar2=0.0,
                op0=AOP.add, op1=AOP.max,
            )
            amask = sp.tile([16, BPC], F32)
            nc.vector.tensor_scalar(
                out=amask[:], in0=aa[:], scalar1=float(ROWS), scalar2=None, op0=AOP.is_lt
            )
            rr = sp.tile([16, BPC], F32)
            nc.vector.tensor_tensor(out=rr[:], in0=aa[:], in1=amask[:], op=AOP.mult)
            # ---- block indices: one selector matmul + diag pick -----------
            # idx8[p, b'] = 128*(rr_r[i(p), b'] - 1) + p1[b']; the -128 is
            # folded into jgboff and invalid rows (rr_r=0) land in the front
            # pad (row 11 of rr = c_3 = clip-mod(p1) = p1 exactly)
            comb2 = pp.tile([128, BPC + 2], F32)
            idx8_ps = comb2[:, 0:BPC]
            nc.tensor.matmul(out=idx8_ps, lhsT=E12c, rhs=rr[:], start=True, stop=True, skip_group_check=True)
            m1 = sp.tile([128, BPC], F32)
            nc.vector.tensor_tensor(out=m1[:], in0=idx8_ps, in1=selmask, op=AOP.mult)
            red = sp.tile([128, 1], F32)
            nc.vector.tensor_reduce(out=red[:], in_=m1[:], axis=AXL.X, op=AOP.add)
            idxs = sp.tile([128, 1], F32)
            nc.vector.tensor_scalar(
                out=idxs[:], in0=red[:], scalar1=jgboff, scalar2=None, op0=AOP.add
            )
            idx128 = sp.tile([128, 1], I32)
            nc.vector.tensor_copy(idx128[:], idxs[:])

            # ---- THE gather: one DMA, 128 x 2KB blocks --------------------
            qgB = sp.tile([128, NS * D], F32R)
            nc.gpsimd.indirect_dma_start(
                out=qgB[:],
                out_offset=None,
                in_=qtab[:].bitcast(F32R),
                in_offset=IndirectOffsetOnAxis(ap=idx128[:, 0:1], axis=0),
            )

            # ---- shift/valid pre-term (overlaps the gather) ---------------
            # pre16 = 512*(rr>0) - (rm1 - p_t)^2/8 per r/c component
            rpos = sp.tile([16, BPC], F32)
            nc.vector.tensor_scalar(
                out=rpos[:], in0=rr[:], scalar1=0.0, scalar2=None, op0=AOP.is_gt
            )
            rm1f = sp.tile([16, BPC], F32)
            nc.vector.tensor_scalar(
                out=rm1f[:], in0=rr[:], scalar1=1.0, scalar2=0.0,
                op0=AOP.subtract, op1=AOP.max,
            )
            rexpd = sp.tile([16, BPC], F32)
            nc.vector.tensor_tensor(out=rexpd[:], in0=rm1f[:], in1=ptb_ps, op=AOP.subtract)
            sq = sp.tile([16, BPC], F32)
            nc.vector.tensor_tensor(out=sq[:], in0=rexpd[:], in1=rexpd[:], op=AOP.mult)
            tsA = sp.tile([16, BPC], F32)
            nc.vector.tensor_scalar(
                out=tsA[:], in0=sq[:], scalar1=-0.125, scalar2=None, op0=AOP.mult
            )
            pre16 = sp.tile([16, BPC], F32)
            nc.vector.scalar_tensor_tensor(
                out=pre16[:], in0=rpos[:], scalar=HALFC, in1=tsA[:],
                op0=AOP.mult, op1=AOP.add,
            )

            # ---- vT[d,b] = sum_c W_a[c,d] c_t[c,b] ------------------------
            vT_ps = pp.tile([D, BPC], F32)
            nc.tensor.matmul(out=vT_ps[:], lhsT=wa0, rhs=ct0, start=True, stop=False)
            nc.tensor.matmul(out=vT_ps[:], lhsT=wa1, rhs=ct1, start=False, stop=True)
            vT_sb = sp.tile([D, BPC], F32R)
            nc.vector.tensor_copy(vT_sb[:], vT_ps[:])

            # ---- scores PSUM: per-batch consts + banded shift/mask --------
            # (1024-scale constants: order-independent, no cancellation loss)
            scores_ps = pp.tile([128, NS * BPC], F32)
            nc.tensor.matmul(
                out=scores_ps[:], lhsT=bsel, rhs=constm8,
                start=True, stop=False, skip_group_check=True,
            )
            for s in range(NS):
                nc.tensor.matmul(
                    out=scores_ps[:, s * BPC : (s + 1) * BPC],
                    lhsT=aL[0:16, 384 + 128 * s : 384 + 128 * (s + 1)],
                    rhs=pre16[:],
                    start=False, stop=False, skip_group_check=True,
                )

            # ---- transpose gathered strips, score matmuls (fp32r) ---------
            qgT_sb = sp.tile([D, NS * 128], F32R)
            for s in range(NS):
                tr_ps = pp.tile([D, 128], F32, tag=f"tr{s % 2}")
                nc.tensor.transpose(tr_ps[:], qgB[:, s * D : (s + 1) * D].bitcast(F32), ident)
                nc.vector.tensor_copy(qgT_sb[:, s * 128 : (s + 1) * 128], tr_ps[:])
                nc.tensor.matmul(
                    out=scores_ps[:, s * BPC : (s + 1) * BPC],
                    lhsT=qgT_sb[:, s * 128 : (s + 1) * 128],
                    rhs=vT_sb[:],
                    start=False, stop=(s == NS - 1), skip_group_check=True,
                )

            # ---- exp straight out of PSUM ---------------------------------
            e_sb = sp.tile([128, NS * BPC], F32R)
            nc.scalar.activation(out=e_sb[:], in_=scores_ps[:], func=ACT.Exp)

            # ---- denominator: S32[8s+b'] = sum_p e[p, 8s+b'] --------------
            S32_ps = comb2[0 : NS * BPC, BPC : BPC + 2]
            nc.tensor.matmul(out=S32_ps, lhsT=e_sb[:], rhs=ones2[:], start=True, stop=True, skip_group_check=True)

            # ---- unnormalized weighted sum (fp32r) ------------------------
            out_ps = pp.tile([BPC, D], F32)
            for s in range(NS):
                nc.tensor.matmul(
                    out=out_ps[:],
                    lhsT=e_sb[:, s * BPC : (s + 1) * BPC],
                    rhs=qgB[:, s * D : (s + 1) * D],
                    start=(s == 0), stop=(s == NS - 1),
                )

            # ---- fold strip sums via a constant matmul, normalize, store --
            S32_sb = sp.tile([NS * BPC, 2], F32)
            nc.vector.tensor_copy(S32_sb[:], S32_ps)
            S8_ps = pp.tile([BPC, 2], F32)
            nc.tensor.matmul(out=S8_ps[:], lhsT=fold8, rhs=S32_sb[:], start=True, stop=True)
            sinv = sp.tile([BPC, 1], F32)
            nc.vector.reciprocal(sinv[:], S8_ps[:, 0:1])
            outf = sp.tile([BPC, D], F32)
            nc.vector.tensor_scalar(
                out=outf[:], in0=out_ps[:], scalar1=sinv[:, 0:1], scalar2=None,
                op0=AOP.mult,
            )
            nc.sync.dma_start(out=out[:], in_=outf[:])

    nc.compile()
    return nc


_CACHE = {}


def _prep_in_maps(q, c_t, W_a, W_p):
    offs = (np.arange(R) - (R // 2 - 1)).astype(np.float32)  # [-3..4]
    p = np.arange(128)
    b_of = p // 16
    i_of = (p % 16) // 2
    jg_of = p % 2

    selc16_np = np.zeros((2, 16), np.float32)
    selc16_np[0, 0:8] = float(H)
    selc16_np[1, 8:16] = float(H)

    oi16_np = offs[np.arange(16) % 8]

    E12c_np = np.zeros((16, 128), np.float32)
    E12c_np[i_of, p] = float(W)     # 128 * rm1_r[i(p), :]
    E12c_np[11, :] = 1.0            # + p1 (rr row 11 = c_3 = p1)

    selmask_np = (np.arange(BPC)[None, :] == b_of[:, None]).astype(np.float32)
    jgboff_np = (b_of * BSTR + PADB - W - 4 + 4 * jg_of).astype(np.float32)

    auxS_np = np.zeros((128, AUXS_W), np.float32)
    auxS_np[:, 16:18] = W_p.T.astype(np.float32)[0:128]
    auxS_np[:, 18:20] = W_p.T.astype(np.float32)[128:256]
    auxS_np[0:2, 20:36] = selc16_np
    auxS_np[0:16, 36] = oi16_np
    auxS_np[0:16, 37:165] = E12c_np
    auxS_np[:, 165:173] = selmask_np
    auxS_np[:, 173] = jgboff_np

    # B_s[q, p] = d(q, i(p)) + d(q, 8 + j(p, s)),  j = 4*jg(p) + s
    auxL_np = np.zeros((128, AUXL_W), np.float32)
    auxL_np[:, 0:128] = W_a.astype(np.float32)[0:128]
    auxL_np[:, 128:256] = W_a.astype(np.float32)[128:256]
    auxL_np[:, 256:384] = np.eye(128, dtype=np.float32)
    for s in range(NS):
        Bs = np.zeros((16, 128), np.float32)
        Bs[i_of, p] += 1.0
        Bs[8 + 4 * jg_of + s, p] += 1.0
        auxL_np[0:16, 384 + 128 * s : 384 + 128 * (s + 1)] = Bs
    bsel_np = (b_of[None, :] == np.arange(8)[:, None]).astype(np.float32)
    auxL_np[0:8, 896:1024] = bsel_np
    constm8_np = np.full((8, NS * BPC), FAKEC, np.float32)
    for r in range(8):
        for s in range(NS):
            constm8_np[r, 8 * s + r] = -BIGC
    auxL_np[0:8, 1024:1056] = constm8_np
    fold8_np = np.zeros((32, 8), np.float32)
    fold8_np[np.arange(32), np.arange(32) % 8] = 1.0
    auxL_np[0:32, 1056:1064] = fold8_np

    in_maps = []
    for c in range(NCORES):
        qs = q[c * BPC : (c + 1) * BPC]  # [BPC, D, H, W]
        qhw_np = np.ascontiguousarray(qs.transpose(0, 2, 3, 1)).reshape(BPC, HW, D)
        qtab_np = np.zeros((BPC, BSTR, D), np.float32)
        qtab_np[:, PADB : PADB + HW, :] = qhw_np
        ctT_np = np.ascontiguousarray(c_t[c * BPC : (c + 1) * BPC].T)  # [CSZ, BPC]
        auxS_c = auxS_np.copy()
        auxS_c[:, 0:8] = ctT_np[0:128]
        auxS_c[:, 8:16] = ctT_np[128:256]
        in_maps.append({
            "qtab": qtab_np.reshape(BPC * BSTR, D),
            "auxS": auxS_c,
            "auxL": auxL_np,
        })
    return in_maps


def run(trace=False, tmpdir=None, **inputs):
    q = np.asarray(inputs["q"], dtype=np.float32)
    c_t = np.asarray(inputs["c_t"], dtype=np.float32)
    W_a = np.asarray(inputs["W_a"], dtype=np.float32)
    W_p = np.asarray(inputs["W_p"], dtype=np.float32)
    if "nc" not in _CACHE:
        _CACHE["nc"] = _build()
    in_maps = _prep_in_maps(q, c_t, W_a, W_p)
    res = _bu.run_bass_kernel_spmd(
        _CACHE["nc"], in_maps, core_ids=list(range(NCORES)), trace=trace,
        tmpdir=tmpdir,
    )
    outp = np.concatenate([r["out"] for r in res.results], axis=0)
    return outp, res


def kernel(**inputs):
    outp, _ = run(trace=False, **inputs)
    return outp


# revision 27
# speedup vs baseline: 1.0301x; 1.0301x over previous
"""LocalAttention2d Bass kernel for 8 Trainium2 NeuronCores.

Strategy: pure data parallel over batch (8 batches/core).  The module only
attends over an 8x8 window of data-dependent spatial positions per batch.
All valid window columns are literally p1+offs (clipping only produces
invalid, masked positions), so each (batch, window-row, col-half) is ONE
contiguous 4-row block of the spatial table.  The kernel computes 128 block
indices on-device and fetches all 512 needed rows with a SINGLE indirect
DMA of 128 x 2KB descriptors from a host-padded [B*(H*W+16), D] table (the
+-8 row pads absorb edge blocks; those positions are invalid and masked).

Layout: partition p = b*16 + i*2 + jg (batch, window-row, col-half); the
index math runs on tiny [16, 8] r/c tiles, is expanded to [128, 1] block
indices by one selector matmul, and the Gaussian shift + validity mask are
folded into the scores PSUM with per-batch-constant and banded matmuls
using the linear mask -1024 + 512*(vr+vc) (exact in f32 at this scale), so
exp() reads PSUM directly.  Softmax is unnormalized: the weighted sum and
the denominator are both matmuls against exp scores, normalized at the end
by a per-partition reciprocal.  Score and weighted-sum matmuls run in
fp32r (single PE pass).

Host-side work is limited to data-INdependent layout prep (transposes of
q / c_t / W_p, constant tables, zero padding); every data-dependent step
(p_t, rounding, window indices, shift, softmax, output) runs on-device.
"""

import sys

import numpy as np

try:
    import concourse.bass_utils as _bu
except ImportError:  # fresh grading dir: fall back to the repo checkout
    sys.path.insert(0, "/opt/trn_rl_repo")
    import concourse.bass_utils as _bu

import concourse.bacc as bacc
import concourse.bass as bass
import concourse.mybir as mybir
import concourse.tile as tile
from concourse.bass import IndirectOffsetOnAxis

B, D, H, W = 64, 128, 128, 128
CSZ = 256
R = 8                     # window rows == cols
NCORES = 8
BPC = B // NCORES         # batches per core
HW = H * W
PADB = 132                # zero rows before each batch (absorbs rr_r=0 blocks)
PADE = 8                  # zero rows after each batch
BSTR = HW + PADB + PADE   # padded batch stride (rows)
ROWS = H + 1              # 129, padded row count in the reference
NS = 4                    # strips (col within 4-row block)
F32 = mybir.dt.float32
F32R = mybir.dt.float32r
I32 = mybir.dt.int32

AOP = mybir.AluOpType
ACT = mybir.ActivationFunctionType
AXL = mybir.AxisListType

BIGC = 1024.0             # mask constants: exact cancellation at 2^10 scale
HALFC = 512.0
FAKEC = -2048.0

# auxS [128, 174] (critical-path constants, first DMA):
#   0:8 ct0 | 8:16 ct1 | 16:18 wp0 | 18:20 wp1 | 20:36 selc16 (parts 0:2)
#   | 36:37 oi16 (parts 0:16) | 37:165 E12c (parts 0:16)
#   | 165:173 selmask | 173:174 jgboff
AUXS_W = 174
# auxL [128, 1064]: 0:128 wa0 | 128:256 wa1 | 256:384 ident
#   | 384:896 B_s x4 (parts 0:16) | 896:1024 bsel (parts 0:8)
#   | 1024:1056 constm8 (parts 0:8) | 1056:1064 fold8 (parts 0:32)
AUXL_W = 1064


def _build():
    nc = bacc.Bacc(
        "TRN2",
        target_bir_lowering=False,
        debug=False,
        num_devices=NCORES,
    )

    qtab = nc.dram_tensor("qtab", [BPC * BSTR, D], F32, kind="ExternalInput")
    auxS = nc.dram_tensor("auxS", [128, AUXS_W], F32, kind="ExternalInput")
    auxL = nc.dram_tensor("auxL", [128, AUXL_W], F32, kind="ExternalInput")
    out = nc.dram_tensor("out", [BPC, D], F32, kind="ExternalOutput")

    with tile.TileContext(nc) as tc:
        with (
            tc.tile_pool(name="sb", bufs=1) as sp,
            tc.tile_pool(name="ps", bufs=1, space="PSUM") as pp,
        ):
            # ---- input DMAs: small critical constants first ---------------
            aS = sp.tile([128, AUXS_W], F32)
            nc.sync.dma_start(out=aS[:], in_=auxS[:])
            aL = sp.tile([128, AUXL_W], F32)
            nc.sync.dma_start(out=aL[:], in_=auxL[:])

            ct0 = aS[:, 0:8]
            ct1 = aS[:, 8:16]
            wp0 = aS[:, 16:18]
            wp1 = aS[:, 18:20]
            selc16 = aS[0:2, 20:36]
            oi16 = aS[0:16, 36:37]
            E12c = aS[0:16, 37:165]
            selmask = aS[:, 165:173]
            jgboff = aS[:, 173:174]
            wa0 = aL[:, 0:128]
            wa1 = aL[:, 128:256]
            ident = aL[:, 256:384]
            bsel = aL[0:8, 896:1024]
            constm8 = aL[0:8, 1024:1056]
            fold8 = aL[0:32, 1056:1064]

            ones2 = sp.tile([128, 2], F32R)
            nc.gpsimd.memset(ones2[:].bitcast(F32), 1.0)

            # ---- prefetch BOTH activation tables (they live in separate
            # table_sel slots) so no 1.3us table load runs after the real
            # sigmoid and delays the sig8 -> PE handoff ---------------------
            dmt = sp.tile([2, 2], F32)
            nc.gpsimd.memset(dmt[:], 0.0)
            dmo = sp.tile([2, 2], F32)
            nc.scalar.activation(out=dmo[:], in_=dmt[:], func=ACT.Sigmoid)
            nc.scalar.activation(out=dmo[:], in_=dmt[:], func=ACT.Exp)

            # ---- p_t: ptT[2,8] = (W_p c_t)^T, spread 128*sigmoid to [16,8] -
            ptcomb = pp.tile([16, 2 * BPC], F32)
            ptT_ps = ptcomb[0:2, BPC : 2 * BPC]
            nc.tensor.matmul(out=ptT_ps, lhsT=wp0, rhs=ct0, start=True, stop=False, skip_group_check=True)
            nc.tensor.matmul(out=ptT_ps, lhsT=wp1, rhs=ct1, start=False, stop=True, skip_group_check=True)
            sig8 = sp.tile([2, BPC], F32)
            nc.scalar.activation(out=sig8[:], in_=ptT_ps, func=ACT.Sigmoid)
            # rows 0:8 = r-part (p_t[:,0]), rows 8:16 = c-part (p_t[:,1])
            ptb_ps = ptcomb[0:16, 0:BPC]
            nc.tensor.matmul(out=ptb_ps, lhsT=selc16, rhs=sig8[:], start=True, stop=True, skip_group_check=True)

            # ---- round via the +-2^23 trick (one op) ----------------------
            prf = sp.tile([16, BPC], F32)
            nc.vector.tensor_scalar(
                out=prf[:], in0=ptb_ps, scalar1=8388608.0, scalar2=8388608.0,
                op0=AOP.add, op1=AOP.subtract,
            )

            # ---- window values: a=max(p+o,0); rr=a*(a<129); rm1=max(rr-1,0)
            aa = sp.tile([16, BPC], F32)
            nc.vector.tensor_scalar(
                out=aa[:], in0=prf[:], scalar1=oi16, scalar2=0.0,
                op0=AOP.add, op1=AOP.max,
            )
            amask = sp.tile([16, BPC], F32)
            nc.vector.tensor_scalar(
                out=amask[:], in0=aa[:], scalar1=float(ROWS), scalar2=None, op0=AOP.is_lt
            )
            rr = sp.tile([16, BPC], F32)
            nc.vector.tensor_tensor(out=rr[:], in0=aa[:], in1=amask[:], op=AOP.mult)
            # ---- block indices: one selector matmul + diag pick -----------
            # idx8[p, b'] = 128*(rr_r[i(p), b'] - 1) + p1[b']; the -128 is
            # folded into jgboff and invalid rows (rr_r=0) land in the front
            # pad (row 11 of rr = c_3 = clip-mod(p1) = p1 exactly)
            comb2 = pp.tile([128, BPC + 2], F32)
            idx8_ps = comb2[:, 0:BPC]
            nc.tensor.matmul(out=idx8_ps, lhsT=E12c, rhs=rr[:], start=True, stop=True, skip_group_check=True)
            m1 = sp.tile([128, BPC], F32)
            nc.vector.tensor_tensor(out=m1[:], in0=idx8_ps, in1=selmask, op=AOP.mult)
            red = sp.tile([128, 1], F32)
            nc.vector.tensor_reduce(out=red[:], in_=m1[:], axis=AXL.X, op=AOP.add)
            idxs = sp.tile([128, 1], F32)
            nc.vector.tensor_scalar(
                out=idxs[:], in0=red[:], scalar1=jgboff, scalar2=None, op0=AOP.add
            )
            idx128 = sp.tile([128, 1], I32)
            nc.vector.tensor_copy(idx128[:], idxs[:])

            # ---- THE gather: one DMA, 128 x 2KB blocks --------------------
            qgB = sp.tile([128, NS * D], F32R)
            nc.gpsimd.indirect_dma_start(
                out=qgB[:],
                out_offset=None,
                in_=qtab[:].bitcast(F32R),
                in_offset=IndirectOffsetOnAxis(ap=idx128[:, 0:1], axis=0),
            )

            # ---- shift/valid pre-term (overlaps the gather) ---------------
            # pre16 = 512*(rr>0) - (rm1 - p_t)^2/8 per r/c component
            rpos = sp.tile([16, BPC], F32)
            nc.vector.tensor_scalar(
                out=rpos[:], in0=rr[:], scalar1=0.0, scalar2=None, op0=AOP.is_gt
            )
            rm1f = sp.tile([16, BPC], F32)
            nc.vector.tensor_scalar(
                out=rm1f[:], in0=rr[:], scalar1=1.0, scalar2=0.0,
                op0=AOP.subtract, op1=AOP.max,
            )
            rexpd = sp.tile([16, BPC], F32)
            nc.vector.tensor_tensor(out=rexpd[:], in0=rm1f[:], in1=ptb_ps, op=AOP.subtract)
            sq = sp.tile([16, BPC], F32)
            nc.vector.tensor_tensor(out=sq[:], in0=rexpd[:], in1=rexpd[:], op=AOP.mult)
            tsA = sp.tile([16, BPC], F32)
            nc.vector.tensor_scalar(
                out=tsA[:], in0=sq[:], scalar1=-0.125, scalar2=None, op0=AOP.mult
            )
            pre16 = sp.tile([16, BPC], F32)
            nc.vector.scalar_tensor_tensor(
                out=pre16[:], in0=rpos[:], scalar=HALFC, in1=tsA[:],
                op0=AOP.mult, op1=AOP.add,
            )

            # ---- vT[d,b] = sum_c W_a[c,d] c_t[c,b] ------------------------
            vT_ps = pp.tile([D, BPC], F32)
            nc.tensor.matmul(out=vT_ps[:], lhsT=wa0, rhs=ct0, start=True, stop=False)
            nc.tensor.matmul(out=vT_ps[:], lhsT=wa1, rhs=ct1, start=False, stop=True)
            vT_sb = sp.tile([D, BPC], F32R)
            nc.vector.tensor_copy(vT_sb[:], vT_ps[:])

            # ---- scores PSUM: per-batch consts + banded shift/mask --------
            # (1024-scale constants: order-independent, no cancellation loss)
            scores_ps = pp.tile([128, NS * BPC], F32)
            nc.tensor.matmul(
                out=scores_ps[:], lhsT=bsel, rhs=constm8,
                start=True, stop=False, skip_group_check=True,
            )
            for s in range(NS):
                nc.tensor.matmul(
                    out=scores_ps[:, s * BPC : (s + 1) * BPC],
                    lhsT=aL[0:16, 384 + 128 * s : 384 + 128 * (s + 1)],
                    rhs=pre16[:],
                    start=False, stop=False, skip_group_check=True,
                )

            # ---- transpose gathered strips, score matmuls (fp32r) ---------
            qgT_sb = sp.tile([D, NS * 128], F32R)
            for s in range(NS):
                tr_ps = pp.tile([D, 128], F32, tag=f"tr{s % 2}")
                nc.tensor.transpose(tr_ps[:], qgB[:, s * D : (s + 1) * D].bitcast(F32), ident)
                nc.vector.tensor_copy(qgT_sb[:, s * 128 : (s + 1) * 128], tr_ps[:])
                nc.tensor.matmul(
                    out=scores_ps[:, s * BPC : (s + 1) * BPC],
                    lhsT=qgT_sb[:, s * 128 : (s + 1) * 128],
                    rhs=vT_sb[:],
                    start=False, stop=(s == NS - 1), skip_group_check=True,
                )

            # ---- exp straight out of PSUM ---------------------------------
            e_sb = sp.tile([128, NS * BPC], F32R)
            nc.scalar.activation(out=e_sb[:], in_=scores_ps[:], func=ACT.Exp)

            # ---- denominator: S32[8s+b'] = sum_p e[p, 8s+b'] --------------
            S32_ps = comb2[0 : NS * BPC, BPC : BPC + 2]
            nc.tensor.matmul(out=S32_ps, lhsT=e_sb[:], rhs=ones2[:], start=True, stop=True, skip_group_check=True)

            # ---- unnormalized weighted sum (fp32r) ------------------------
            out_ps = pp.tile([BPC, D], F32)
            for s in range(NS):
                nc.tensor.matmul(
                    out=out_ps[:],
                    lhsT=e_sb[:, s * BPC : (s + 1) * BPC],
                    rhs=qgB[:, s * D : (s + 1) * D],
                    start=(s == 0), stop=(s == NS - 1),
                )

            # ---- fold strip sums via a constant matmul, normalize, store --
            S32_sb = sp.tile([NS * BPC, 2], F32)
            nc.vector.tensor_copy(S32_sb[:], S32_ps)
            S8_ps = pp.tile([BPC, 2], F32)
            nc.tensor.matmul(out=S8_ps[:], lhsT=fold8, rhs=S32_sb[:], start=True, stop=True)
            sinv = sp.tile([BPC, 1], F32)
            nc.vector.reciprocal(sinv[:], S8_ps[:, 0:1])
            outf = sp.tile([BPC, D], F32)
            nc.vector.tensor_scalar(
                out=outf[:], in0=out_ps[:], scalar1=sinv[:, 0:1], scalar2=None,
                op0=AOP.mult,
            )
            nc.sync.dma_start(out=out[:], in_=outf[:])

    nc.compile()
    return nc


_CACHE = {}


def _prep_in_maps(q, c_t, W_a, W_p):
    offs = (np.arange(R) - (R // 2 - 1)).astype(np.float32)  # [-3..4]
    p = np.arange(128)
    b_of = p // 16
    i_of = (p % 16) // 2
    jg_of = p % 2

    selc16_np = np.zeros((2, 16), np.float32)
    selc16_np[0, 0:8] = float(H)
    selc16_np[1, 8:16] = float(H)

    oi16_np = offs[np.arange(16) % 8]

    E12c_np = np.zeros((16, 128), np.float32)
    E12c_np[i_of, p] = float(W)     # 128 * rm1_r[i(p), :]
    E12c_np[11, :] = 1.0            # + p1 (rr row 11 = c_3 = p1)

    selmask_np = (np.arange(BPC)[None, :] == b_of[:, None]).astype(np.float32)
    jgboff_np = (b_of * BSTR + PADB - W - 4 + 4 * jg_of).astype(np.float32)

    auxS_np = np.zeros((128, AUXS_W), np.float32)
    auxS_np[:, 16:18] = W_p.T.astype(np.float32)[0:128]
    auxS_np[:, 18:20] = W_p.T.astype(np.float32)[128:256]
    auxS_np[0:2, 20:36] = selc16_np
    auxS_np[0:16, 36] = oi16_np
    auxS_np[0:16, 37:165] = E12c_np
    auxS_np[:, 165:173] = selmask_np
    auxS_np[:, 173] = jgboff_np

    # B_s[q, p] = d(q, i(p)) + d(q, 8 + j(p, s)),  j = 4*jg(p) + s
    auxL_np = np.zeros((128, AUXL_W), np.float32)
    auxL_np[:, 0:128] = W_a.astype(np.float32)[0:128]
    auxL_np[:, 128:256] = W_a.astype(np.float32)[128:256]
    auxL_np[:, 256:384] = np.eye(128, dtype=np.float32)
    for s in range(NS):
        Bs = np.zeros((16, 128), np.float32)
        Bs[i_of, p] += 1.0
        Bs[8 + 4 * jg_of + s, p] += 1.0
        auxL_np[0:16, 384 + 128 * s : 384 + 128 * (s + 1)] = Bs
    bsel_np = (b_of[None, :] == np.arange(8)[:, None]).astype(np.float32)
    auxL_np[0:8, 896:1024] = bsel_np
    constm8_np = np.full((8, NS * BPC), FAKEC, np.float32)
    for r in range(8):
        for s in range(NS):
            constm8_np[r, 8 * s + r] = -BIGC
    auxL_np[0:8, 1024:1056] = constm8_np
    fold8_np = np.zeros((32, 8), np.float32)
    fold8_np[np.arange(32), np.arange(32) % 8] = 1.0
    auxL_np[0:32, 1056:1064] = fold8_np

    in_maps = []
    for c in range(NCORES):
        qs = q[c * BPC : (c + 1) * BPC]  # [BPC, D, H, W]
        qhw_np = np.ascontiguousarray(qs.transpose(0, 2, 3, 1)).reshape(BPC, HW, D)
        qtab_np = np.zeros((BPC, BSTR, D), np.float32)
        qtab_np[:, PADB : PADB + HW, :] = qhw_np
        ctT_np = np.ascontiguousarray(c_t[c * BPC : (c + 1) * BPC].T)  # [CSZ, BPC]
        auxS_c = auxS_np.copy()
        auxS_c[:, 0:8] = ctT_np[0:128]
        auxS_c[:, 8:16] = ctT_np[128:256]
        in_maps.append({
            "qtab": qtab_np.reshape(BPC * BSTR, D),
            "auxS": auxS_c,
            "auxL": auxL_np,
        })
    return in_maps


def run(trace=False, tmpdir=None, **inputs):
    q = np.asarray(inputs["q"], dtype=np.float32)
    c_t = np.asarray(inputs["c_t"], dtype=np.float32)
    W_a = np.asarray(inputs["W_a"], dtype=np.float32)
    W_p = np.asarray(inputs["W_p"], dtype=np.float32)
    if "nc" not in _CACHE:
        _CACHE["nc"] = _build()
    in_maps = _prep_in_maps(q, c_t, W_a, W_p)
    res = _bu.run_bass_kernel_spmd(
        _CACHE["nc"], in_maps, core_ids=list(range(NCORES)), trace=trace,
        tmpdir=tmpdir,
    )
    outp = np.concatenate([r["out"] for r in res.results], axis=0)
    return outp, res


def kernel(**inputs):
    outp, _ = run(trace=False, **inputs)
    return outp


# revision 28
# speedup vs baseline: 1.0565x; 1.0256x over previous
"""LocalAttention2d Bass kernel for 8 Trainium2 NeuronCores.

Strategy: pure data parallel over batch (8 batches/core).  The module only
attends over an 8x8 window of data-dependent spatial positions per batch.
All valid window columns are literally p1+offs (clipping only produces
invalid, masked positions), so each (batch, window-row, col-half) is ONE
contiguous 4-row block of the spatial table.  The kernel computes 128 block
indices on-device and fetches all 512 needed rows with a SINGLE indirect
DMA of 128 x 2KB descriptors from a host-padded [B*(H*W+16), D] table (the
+-8 row pads absorb edge blocks; those positions are invalid and masked).

Layout: partition p = b*16 + i*2 + jg (batch, window-row, col-half); the
index math runs on tiny [16, 8] r/c tiles, is expanded to [128, 1] block
indices by one selector matmul, and the Gaussian shift + validity mask are
folded into the scores PSUM with per-batch-constant and banded matmuls
using the linear mask -1024 + 512*(vr+vc) (exact in f32 at this scale), so
exp() reads PSUM directly.  Softmax is unnormalized: the weighted sum and
the denominator are both matmuls against exp scores, normalized at the end
by a per-partition reciprocal.  Score and weighted-sum matmuls run in
fp32r (single PE pass).

Host-side work is limited to data-INdependent layout prep (transposes of
q / c_t / W_p, constant tables, zero padding); every data-dependent step
(p_t, rounding, window indices, shift, softmax, output) runs on-device.
"""

import sys

import numpy as np

try:
    import concourse.bass_utils as _bu
except ImportError:  # fresh grading dir: fall back to the repo checkout
    sys.path.insert(0, "/opt/trn_rl_repo")
    import concourse.bass_utils as _bu

import concourse.bacc as bacc
import concourse.bass as bass
import concourse.mybir as mybir
import concourse.tile as tile
from concourse.bass import IndirectOffsetOnAxis

B, D, H, W = 64, 128, 128, 128
CSZ = 256
R = 8                     # window rows == cols
NCORES = 8
BPC = B // NCORES         # batches per core
HW = H * W
PADB = 132                # zero rows before each batch (absorbs rr_r=0 blocks)
PADE = 8                  # zero rows after each batch
BSTR = HW + PADB + PADE   # padded batch stride (rows)
ROWS = H + 1              # 129, padded row count in the reference
NS = 4                    # strips (col within 4-row block)
F32 = mybir.dt.float32
F32R = mybir.dt.float32r
I32 = mybir.dt.int32

AOP = mybir.AluOpType
ACT = mybir.ActivationFunctionType
AXL = mybir.AxisListType

BIGC = 1024.0             # mask constants: exact cancellation at 2^10 scale
HALFC = 512.0
FAKEC = -2048.0

# auxS [128, 186] (critical-path constants, first DMA):
#   0:8 ct0 | 8:16 ct1 | 16:32 WPbig0 | 32:48 WPbig1 (r/c-duplicated W_p)
#   | 48:49 oi16m (parts 0:16, offs - 2^23) | 49:177 E12c (parts 0:16)
#   | 177:185 selmask | 185:186 jgboff
AUXS_W = 186
# auxL [128, 1064]: 0:128 wa0 | 128:256 wa1 | 256:384 ident
#   | 384:896 B_s x4 (parts 0:16) | 896:1024 bsel (parts 0:8)
#   | 1024:1056 constm8 (parts 0:8) | 1056:1064 fold8 (parts 0:32)
AUXL_W = 1064


def _build():
    nc = bacc.Bacc(
        "TRN2",
        target_bir_lowering=False,
        debug=False,
        num_devices=NCORES,
    )

    qtab = nc.dram_tensor("qtab", [BPC * BSTR, D], F32, kind="ExternalInput")
    auxS = nc.dram_tensor("auxS", [128, AUXS_W], F32, kind="ExternalInput")
    auxL = nc.dram_tensor("auxL", [128, AUXL_W], F32, kind="ExternalInput")
    out = nc.dram_tensor("out", [BPC, D], F32, kind="ExternalOutput")

    with tile.TileContext(nc) as tc:
        with (
            tc.tile_pool(name="sb", bufs=1) as sp,
            tc.tile_pool(name="ps", bufs=1, space="PSUM") as pp,
        ):
            # ---- input DMAs: small critical constants first ---------------
            aS = sp.tile([128, AUXS_W], F32)
            nc.sync.dma_start(out=aS[:], in_=auxS[:])
            aL = sp.tile([128, AUXL_W], F32)
            nc.sync.dma_start(out=aL[:], in_=auxL[:])

            ct0 = aS[:, 0:8]
            ct1 = aS[:, 8:16]
            wpb0 = aS[:, 16:32]
            wpb1 = aS[:, 32:48]
            oi16m = aS[0:16, 48:49]
            E12c = aS[0:16, 49:177]
            selmask = aS[:, 177:185]
            jgboff = aS[:, 185:186]
            wa0 = aL[:, 0:128]
            wa1 = aL[:, 128:256]
            ident = aL[:, 256:384]
            bsel = aL[0:8, 896:1024]
            constm8 = aL[0:8, 1024:1056]
            fold8 = aL[0:32, 1056:1064]

            ones2 = sp.tile([128, 2], F32R)
            nc.gpsimd.memset(ones2[:].bitcast(F32), 1.0)

            # ---- p_t logits straight to [16,8] via duplicated W_p weights;
            # rows 0:8 = r-part (p_t[:,0]), rows 8:16 = c-part (p_t[:,1]) ----
            ptcomb = pp.tile([16, BPC], F32)
            nc.tensor.matmul(out=ptcomb[:], lhsT=wpb0, rhs=ct0, start=True, stop=False)
            nc.tensor.matmul(out=ptcomb[:], lhsT=wpb1, rhs=ct1, start=False, stop=True)
            sig16 = sp.tile([16, BPC], F32)
            nc.scalar.activation(out=sig16[:], in_=ptcomb[:], func=ACT.Sigmoid)

            # ---- 128*sig + 2^23 rounds to integer; the -2^23 is folded
            # into oi16m = offs - 2^23 -----------------------------------
            prfraw = sp.tile([16, BPC], F32)
            nc.vector.tensor_scalar(
                out=prfraw[:], in0=sig16[:], scalar1=128.0, scalar2=8388608.0,
                op0=AOP.mult, op1=AOP.add,
            )

            # ---- window values: a=max(p+o,0); rr=a*(a<129); rm1=max(rr-1,0)
            aa = sp.tile([16, BPC], F32)
            nc.vector.tensor_scalar(
                out=aa[:], in0=prfraw[:], scalar1=oi16m, scalar2=0.0,
                op0=AOP.add, op1=AOP.max,
            )
            amask = sp.tile([16, BPC], F32)
            nc.vector.tensor_scalar(
                out=amask[:], in0=aa[:], scalar1=float(ROWS), scalar2=None, op0=AOP.is_lt
            )
            rr = sp.tile([16, BPC], F32)
            nc.vector.tensor_tensor(out=rr[:], in0=aa[:], in1=amask[:], op=AOP.mult)
            # ---- block indices: one selector matmul + diag pick -----------
            # idx8[p, b'] = 128*(rr_r[i(p), b'] - 1) + p1[b']; the -128 is
            # folded into jgboff and invalid rows (rr_r=0) land in the front
            # pad (row 11 of rr = c_3 = clip-mod(p1) = p1 exactly)
            comb2 = pp.tile([128, BPC + 2], F32)
            idx8_ps = comb2[:, 0:BPC]
            nc.tensor.matmul(out=idx8_ps, lhsT=E12c, rhs=rr[:], start=True, stop=True, skip_group_check=True)
            m1 = sp.tile([128, BPC], F32)
            nc.vector.tensor_tensor(out=m1[:], in0=idx8_ps, in1=selmask, op=AOP.mult)
            red = sp.tile([128, 1], F32)
            nc.vector.tensor_reduce(out=red[:], in_=m1[:], axis=AXL.X, op=AOP.add)
            idxs = sp.tile([128, 1], F32)
            nc.vector.tensor_scalar(
                out=idxs[:], in0=red[:], scalar1=jgboff, scalar2=None, op0=AOP.add
            )
            idx128 = sp.tile([128, 1], I32)
            nc.vector.tensor_copy(idx128[:], idxs[:])

            # ---- THE gather: one DMA, 128 x 2KB blocks --------------------
            qgB = sp.tile([128, NS * D], F32R)
            nc.gpsimd.indirect_dma_start(
                out=qgB[:],
                out_offset=None,
                in_=qtab[:].bitcast(F32R),
                in_offset=IndirectOffsetOnAxis(ap=idx128[:, 0:1], axis=0),
            )

            # ---- shift/valid pre-term (overlaps the gather) ---------------
            # pre16 = 512*(rr>0) - (rm1 - p_t)^2/8 per r/c component
            rpos = sp.tile([16, BPC], F32)
            nc.vector.tensor_scalar(
                out=rpos[:], in0=rr[:], scalar1=0.0, scalar2=None, op0=AOP.is_gt
            )
            rm1f = sp.tile([16, BPC], F32)
            nc.vector.tensor_scalar(
                out=rm1f[:], in0=rr[:], scalar1=1.0, scalar2=0.0,
                op0=AOP.subtract, op1=AOP.max,
            )
            rexpd = sp.tile([16, BPC], F32)
            nc.vector.scalar_tensor_tensor(
                out=rexpd[:], in0=sig16[:], scalar=-128.0, in1=rm1f[:],
                op0=AOP.mult, op1=AOP.add,
            )
            sq = sp.tile([16, BPC], F32)
            nc.vector.tensor_tensor(out=sq[:], in0=rexpd[:], in1=rexpd[:], op=AOP.mult)
            tsA = sp.tile([16, BPC], F32)
            nc.vector.tensor_scalar(
                out=tsA[:], in0=sq[:], scalar1=-0.125, scalar2=None, op0=AOP.mult
            )
            pre16 = sp.tile([16, BPC], F32)
            nc.vector.scalar_tensor_tensor(
                out=pre16[:], in0=rpos[:], scalar=HALFC, in1=tsA[:],
                op0=AOP.mult, op1=AOP.add,
            )

            # ---- vT[d,b] = sum_c W_a[c,d] c_t[c,b] ------------------------
            vT_ps = pp.tile([D, BPC], F32)
            nc.tensor.matmul(out=vT_ps[:], lhsT=wa0, rhs=ct0, start=True, stop=False)
            nc.tensor.matmul(out=vT_ps[:], lhsT=wa1, rhs=ct1, start=False, stop=True)
            vT_sb = sp.tile([D, BPC], F32R)
            nc.vector.tensor_copy(vT_sb[:], vT_ps[:])

            # ---- scores PSUM: per-batch consts + banded shift/mask --------
            # (1024-scale constants: order-independent, no cancellation loss)
            scores_ps = pp.tile([128, NS * BPC], F32)
            nc.tensor.matmul(
                out=scores_ps[:], lhsT=bsel, rhs=constm8,
                start=True, stop=False, skip_group_check=True,
            )
            for s in range(NS):
                nc.tensor.matmul(
                    out=scores_ps[:, s * BPC : (s + 1) * BPC],
                    lhsT=aL[0:16, 384 + 128 * s : 384 + 128 * (s + 1)],
                    rhs=pre16[:],
                    start=False, stop=False, skip_group_check=True,
                )

            # ---- transpose gathered strips, score matmuls (fp32r) ---------
            qgT_sb = sp.tile([D, NS * 128], F32R)
            for s in range(NS):
                tr_ps = pp.tile([D, 128], F32, tag=f"tr{s % 2}")
                nc.tensor.transpose(tr_ps[:], qgB[:, s * D : (s + 1) * D].bitcast(F32), ident)
                nc.vector.tensor_copy(qgT_sb[:, s * 128 : (s + 1) * 128], tr_ps[:])
                nc.tensor.matmul(
                    out=scores_ps[:, s * BPC : (s + 1) * BPC],
                    lhsT=qgT_sb[:, s * 128 : (s + 1) * 128],
                    rhs=vT_sb[:],
                    start=False, stop=(s == NS - 1), skip_group_check=True,
                )

            # ---- exp straight out of PSUM ---------------------------------
            e_sb = sp.tile([128, NS * BPC], F32R)
            nc.scalar.activation(out=e_sb[:], in_=scores_ps[:], func=ACT.Exp)

            # ---- denominator: S32[8s+b'] = sum_p e[p, 8s+b'] --------------
            S32_ps = comb2[0 : NS * BPC, BPC : BPC + 2]
            nc.tensor.matmul(out=S32_ps, lhsT=e_sb[:], rhs=ones2[:], start=True, stop=True, skip_group_check=True)

            # ---- unnormalized weighted sum (fp32r) ------------------------
            out_ps = pp.tile([BPC, D], F32)
            for s in range(NS):
                nc.tensor.matmul(
                    out=out_ps[:],
                    lhsT=e_sb[:, s * BPC : (s + 1) * BPC],
                    rhs=qgB[:, s * D : (s + 1) * D],
                    start=(s == 0), stop=(s == NS - 1),
                )

            # ---- fold strip sums via a constant matmul, normalize, store --
            S32_sb = sp.tile([NS * BPC, 2], F32)
            nc.vector.tensor_copy(S32_sb[:], S32_ps)
            S8_ps = pp.tile([BPC, 2], F32)
            nc.tensor.matmul(out=S8_ps[:], lhsT=fold8, rhs=S32_sb[:], start=True, stop=True)
            sinv = sp.tile([BPC, 1], F32)
            nc.vector.reciprocal(sinv[:], S8_ps[:, 0:1])
            outf = sp.tile([BPC, D], F32)
            nc.vector.tensor_scalar(
                out=outf[:], in0=out_ps[:], scalar1=sinv[:, 0:1], scalar2=None,
                op0=AOP.mult,
            )
            nc.sync.dma_start(out=out[:], in_=outf[:])

    nc.compile()
    return nc


_CACHE = {}


def _prep_in_maps(q, c_t, W_a, W_p):
    offs = (np.arange(R) - (R // 2 - 1)).astype(np.float32)  # [-3..4]
    p = np.arange(128)
    b_of = p // 16
    i_of = (p % 16) // 2
    jg_of = p % 2

    WPT = W_p.T.astype(np.float32)          # [256, 2]
    WPbig = np.zeros((256, 16), np.float32)
    WPbig[:, 0:8] = WPT[:, 0:1]
    WPbig[:, 8:16] = WPT[:, 1:2]

    oi16m_np = offs[np.arange(16) % 8] - 8388608.0

    E12c_np = np.zeros((16, 128), np.float32)
    E12c_np[i_of, p] = float(W)     # 128 * rm1_r[i(p), :]
    E12c_np[11, :] = 1.0            # + p1 (rr row 11 = c_3 = p1)

    selmask_np = (np.arange(BPC)[None, :] == b_of[:, None]).astype(np.float32)
    jgboff_np = (b_of * BSTR + PADB - W - 4 + 4 * jg_of).astype(np.float32)

    auxS_np = np.zeros((128, AUXS_W), np.float32)
    auxS_np[:, 16:32] = WPbig[0:128]
    auxS_np[:, 32:48] = WPbig[128:256]
    auxS_np[0:16, 48] = oi16m_np
    auxS_np[0:16, 49:177] = E12c_np
    auxS_np[:, 177:185] = selmask_np
    auxS_np[:, 185] = jgboff_np

    # B_s[q, p] = d(q, i(p)) + d(q, 8 + j(p, s)),  j = 4*jg(p) + s
    auxL_np = np.zeros((128, AUXL_W), np.float32)
    auxL_np[:, 0:128] = W_a.astype(np.float32)[0:128]
    auxL_np[:, 128:256] = W_a.astype(np.float32)[128:256]
    auxL_np[:, 256:384] = np.eye(128, dtype=np.float32)
    for s in range(NS):
        Bs = np.zeros((16, 128), np.float32)
        Bs[i_of, p] += 1.0
        Bs[8 + 4 * jg_of + s, p] += 1.0
        auxL_np[0:16, 384 + 128 * s : 384 + 128 * (s + 1)] = Bs
    bsel_np = (b_of[None, :] == np.arange(8)[:, None]).astype(np.float32)
    auxL_np[0:8, 896:1024] = bsel_np
    constm8_np = np.full((8, NS * BPC), FAKEC, np.float32)
    for r in range(8):
        for s in range(NS):
            constm8_np[r, 8 * s + r] = -BIGC
    auxL_np[0:8, 1024:1056] = constm8_np
    fold8_np = np.zeros((32, 8), np.float32)
    fold8_np[np.arange(32), np.arange(32) % 8] = 1.0
    auxL_np[0:32, 1056:1064] = fold8_np

    in_maps = []
    for c in range(NCORES):
        qs = q[c * BPC : (c + 1) * BPC]  # [BPC, D, H, W]
        qhw_np = np.ascontiguousarray(qs.transpose(0, 2, 3, 1)).reshape(BPC, HW, D)
        qtab_np = np.zeros((BPC, BSTR, D), np.float32)
        qtab_np[:, PADB : PADB + HW, :] = qhw_np
        ctT_np = np.ascontiguousarray(c_t[c * BPC : (c + 1) * BPC].T)  # [CSZ, BPC]
        auxS_c = auxS_np.copy()
        auxS_c[:, 0:8] = ctT_np[0:128]
        auxS_c[:, 8:16] = ctT_np[128:256]
        in_maps.append({
            "qtab": qtab_np.reshape(BPC * BSTR, D),
            "auxS": auxS_c,
            "auxL": auxL_np,
        })
    return in_maps


def run(trace=False, tmpdir=None, **inputs):
    q = np.asarray(inputs["q"], dtype=np.float32)
    c_t = np.asarray(inputs["c_t"], dtype=np.float32)
    W_a = np.asarray(inputs["W_a"], dtype=np.float32)
    W_p = np.asarray(inputs["W_p"], dtype=np.float32)
    if "nc" not in _CACHE:
        _CACHE["nc"] = _build()
    in_maps = _prep_in_maps(q, c_t, W_a, W_p)
    res = _bu.run_bass_kernel_spmd(
        _CACHE["nc"], in_maps, core_ids=list(range(NCORES)), trace=trace,
        tmpdir=tmpdir,
    )
    outp = np.concatenate([r["out"] for r in res.results], axis=0)
    return outp, res


def kernel(**inputs):
    outp, _ = run(trace=False, **inputs)
    return outp


# revision 29
# speedup vs baseline: 1.0628x; 1.0060x over previous
"""LocalAttention2d Bass kernel for 8 Trainium2 NeuronCores.

Strategy: pure data parallel over batch (8 batches/core).  The module only
attends over an 8x8 window of data-dependent spatial positions per batch.
All valid window columns are literally p1+offs (clipping only produces
invalid, masked positions), so each (batch, window-row, col-half) is ONE
contiguous 4-row block of the spatial table.  The kernel computes 128 block
indices on-device and fetches all 512 needed rows with a SINGLE indirect
DMA of 128 x 2KB descriptors from a host-padded [B*(H*W+16), D] table (the
+-8 row pads absorb edge blocks; those positions are invalid and masked).

Layout: partition p = b*16 + i*2 + jg (batch, window-row, col-half); the
index math runs on tiny [16, 8] r/c tiles, is expanded to [128, 1] block
indices by one selector matmul, and the Gaussian shift + validity mask are
folded into the scores PSUM with per-batch-constant and banded matmuls
using the linear mask -1024 + 512*(vr+vc) (exact in f32 at this scale), so
exp() reads PSUM directly.  Softmax is unnormalized: the weighted sum and
the denominator are both matmuls against exp scores, normalized at the end
by a per-partition reciprocal.  Score and weighted-sum matmuls run in
fp32r (single PE pass).

Host-side work is limited to data-INdependent layout prep (transposes of
q / c_t / W_p, constant tables, zero padding); every data-dependent step
(p_t, rounding, window indices, shift, softmax, output) runs on-device.
"""

import sys

import numpy as np

try:
    import concourse.bass_utils as _bu
except ImportError:  # fresh grading dir: fall back to the repo checkout
    sys.path.insert(0, "/opt/trn_rl_repo")
    import concourse.bass_utils as _bu

import concourse.bacc as bacc
import concourse.bass as bass
import concourse.mybir as mybir
import concourse.tile as tile
from concourse.bass import IndirectOffsetOnAxis

B, D, H, W = 64, 128, 128, 128
CSZ = 256
R = 8                     # window rows == cols
NCORES = 8
BPC = B // NCORES         # batches per core
HW = H * W
PADB = 132                # zero rows before each batch (absorbs rr_r=0 blocks)
PADE = 8                  # zero rows after each batch
BSTR = HW + PADB + PADE   # padded batch stride (rows)
ROWS = H + 1              # 129, padded row count in the reference
NS = 4                    # strips (col within 4-row block)
F32 = mybir.dt.float32
F32R = mybir.dt.float32r
I32 = mybir.dt.int32

AOP = mybir.AluOpType
ACT = mybir.ActivationFunctionType
AXL = mybir.AxisListType

BIGC = 1024.0             # mask constants: exact cancellation at 2^10 scale
HALFC = 512.0
FAKEC = -2048.0

# auxS [128, 186] (critical-path constants, first DMA):
#   0:8 ct0 | 8:16 ct1 | 16:32 WPbig0 | 32:48 WPbig1 (r/c-duplicated W_p)
#   | 48:49 oi16m (parts 0:16, offs - 2^23) | 49:177 E12c (parts 0:16)
#   | 177:185 selmask | 185:186 jgboff
AUXS_W = 186
# auxL [128, 1064]: 0:128 wa0 | 128:256 wa1 | 256:384 ident
#   | 384:896 B_s x4 (parts 0:16) | 896:1024 bsel (parts 0:8)
#   | 1024:1056 constm8 (parts 0:8) | 1056:1064 fold8 (parts 0:32)
AUXL_W = 1064


def _build():
    nc = bacc.Bacc(
        "TRN2",
        target_bir_lowering=False,
        debug=False,
        num_devices=NCORES,
    )

    qtab = nc.dram_tensor("qtab", [BPC * BSTR, D], F32, kind="ExternalInput")
    auxS = nc.dram_tensor("auxS", [128, AUXS_W], F32, kind="ExternalInput")
    auxL = nc.dram_tensor("auxL", [128, AUXL_W], F32, kind="ExternalInput")
    out = nc.dram_tensor("out", [BPC, D], F32, kind="ExternalOutput")

    with tile.TileContext(nc) as tc:
        with (
            tc.tile_pool(name="sb", bufs=1) as sp,
            tc.tile_pool(name="ps", bufs=1, space="PSUM") as pp,
        ):
            # ---- input DMAs: small critical constants first ---------------
            aS = sp.tile([128, AUXS_W], F32)
            nc.sync.dma_start(out=aS[:], in_=auxS[:])
            aL = sp.tile([128, AUXL_W], F32)
            nc.sync.dma_start(out=aL[:], in_=auxL[:])

            ct0 = aS[:, 0:8]
            ct1 = aS[:, 8:16]
            wpb0 = aS[:, 16:32]
            wpb1 = aS[:, 32:48]
            oi16m = aS[0:16, 48:49]
            E12c = aS[0:16, 49:177]
            selmask = aS[:, 177:185]
            jgboff = aS[:, 185:186]
            wa0 = aL[:, 0:128]
            wa1 = aL[:, 128:256]
            ident = aL[:, 256:384]
            bsel = aL[0:8, 896:1024]
            constm8 = aL[0:8, 1024:1056]
            fold8 = aL[0:32, 1056:1064]

            ones2 = sp.tile([128, 2], F32R)
            nc.gpsimd.memset(ones2[:].bitcast(F32), 1.0)

            # ---- p_t logits straight to [16,8] via duplicated W_p weights;
            # rows 0:8 = r-part (p_t[:,0]), rows 8:16 = c-part (p_t[:,1]) ----
            ptcomb = pp.tile([16, BPC], F32)
            nc.tensor.matmul(out=ptcomb[:], lhsT=wpb0, rhs=ct0, start=True, stop=False)
            nc.tensor.matmul(out=ptcomb[:], lhsT=wpb1, rhs=ct1, start=False, stop=True)
            # sigmoid via exp so the kernel uses ONE activation table,
            # loaded once before any data arrives (a second table load
            # would serialize the scalar->vector handoff semaphore)
            e16 = sp.tile([16, BPC], F32)
            nc.scalar.activation(out=e16[:], in_=ptcomb[:], func=ACT.Exp, scale=-1.0)
            one16 = sp.tile([16, BPC], F32)
            nc.vector.tensor_scalar(
                out=one16[:], in0=e16[:], scalar1=1.0, scalar2=None, op0=AOP.add
            )
            sig16 = sp.tile([16, BPC], F32)
            nc.vector.reciprocal(sig16[:], one16[:])

            # ---- 128*sig + 2^23 rounds to integer; the -2^23 is folded
            # into oi16m = offs - 2^23 -----------------------------------
            prfraw = sp.tile([16, BPC], F32)
            nc.vector.tensor_scalar(
                out=prfraw[:], in0=sig16[:], scalar1=128.0, scalar2=8388608.0,
                op0=AOP.mult, op1=AOP.add,
            )

            # ---- window values: a=max(p+o,0); rr=a*(a<129); rm1=max(rr-1,0)
            aa = sp.tile([16, BPC], F32)
            nc.vector.tensor_scalar(
                out=aa[:], in0=prfraw[:], scalar1=oi16m, scalar2=0.0,
                op0=AOP.add, op1=AOP.max,
            )
            amask = sp.tile([16, BPC], F32)
            nc.vector.tensor_scalar(
                out=amask[:], in0=aa[:], scalar1=float(ROWS), scalar2=None, op0=AOP.is_lt
            )
            rr = sp.tile([16, BPC], F32)
            nc.vector.tensor_tensor(out=rr[:], in0=aa[:], in1=amask[:], op=AOP.mult)
            # ---- block indices: one selector matmul + diag pick -----------
            # idx8[p, b'] = 128*(rr_r[i(p), b'] - 1) + p1[b']; the -128 is
            # folded into jgboff and invalid rows (rr_r=0) land in the front
            # pad (row 11 of rr = c_3 = clip-mod(p1) = p1 exactly)
            comb2 = pp.tile([128, BPC + 2], F32)
            idx8_ps = comb2[:, 0:BPC]
            nc.tensor.matmul(out=idx8_ps, lhsT=E12c, rhs=rr[:], start=True, stop=True, skip_group_check=True)
            m1 = sp.tile([128, BPC], F32)
            nc.vector.tensor_tensor(out=m1[:], in0=idx8_ps, in1=selmask, op=AOP.mult)
            red = sp.tile([128, 1], F32)
            nc.vector.tensor_reduce(out=red[:], in_=m1[:], axis=AXL.X, op=AOP.add)
            idxs = sp.tile([128, 1], F32)
            nc.vector.tensor_scalar(
                out=idxs[:], in0=red[:], scalar1=jgboff, scalar2=None, op0=AOP.add
            )
            idx128 = sp.tile([128, 1], I32)
            nc.vector.tensor_copy(idx128[:], idxs[:])

            # ---- THE gather: one DMA, 128 x 2KB blocks --------------------
            qgB = sp.tile([128, NS * D], F32R)
            nc.gpsimd.indirect_dma_start(
                out=qgB[:],
                out_offset=None,
                in_=qtab[:].bitcast(F32R),
                in_offset=IndirectOffsetOnAxis(ap=idx128[:, 0:1], axis=0),
            )

            # ---- shift/valid pre-term (overlaps the gather) ---------------
            # pre16 = 512*(rr>0) - (rm1 - p_t)^2/8 per r/c component
            rpos = sp.tile([16, BPC], F32)
            nc.vector.tensor_scalar(
                out=rpos[:], in0=rr[:], scalar1=0.0, scalar2=None, op0=AOP.is_gt
            )
            rm1f = sp.tile([16, BPC], F32)
            nc.vector.tensor_scalar(
                out=rm1f[:], in0=rr[:], scalar1=1.0, scalar2=0.0,
                op0=AOP.subtract, op1=AOP.max,
            )
            rexpd = sp.tile([16, BPC], F32)
            nc.vector.scalar_tensor_tensor(
                out=rexpd[:], in0=sig16[:], scalar=-128.0, in1=rm1f[:],
                op0=AOP.mult, op1=AOP.add,
            )
            sq = sp.tile([16, BPC], F32)
            nc.vector.tensor_tensor(out=sq[:], in0=rexpd[:], in1=rexpd[:], op=AOP.mult)
            tsA = sp.tile([16, BPC], F32)
            nc.vector.tensor_scalar(
                out=tsA[:], in0=sq[:], scalar1=-0.125, scalar2=None, op0=AOP.mult
            )
            pre16 = sp.tile([16, BPC], F32)
            nc.vector.scalar_tensor_tensor(
                out=pre16[:], in0=rpos[:], scalar=HALFC, in1=tsA[:],
                op0=AOP.mult, op1=AOP.add,
            )

            # ---- vT[d,b] = sum_c W_a[c,d] c_t[c,b] ------------------------
            vT_ps = pp.tile([D, BPC], F32)
            nc.tensor.matmul(out=vT_ps[:], lhsT=wa0, rhs=ct0, start=True, stop=False)
            nc.tensor.matmul(out=vT_ps[:], lhsT=wa1, rhs=ct1, start=False, stop=True)
            vT_sb = sp.tile([D, BPC], F32R)
            nc.vector.tensor_copy(vT_sb[:], vT_ps[:])

            # ---- scores PSUM: per-batch consts + banded shift/mask --------
            # (1024-scale constants: order-independent, no cancellation loss)
            scores_ps = pp.tile([128, NS * BPC], F32)
            nc.tensor.matmul(
                out=scores_ps[:], lhsT=bsel, rhs=constm8,
                start=True, stop=False, skip_group_check=True,
            )
            for s in range(NS):
                nc.tensor.matmul(
                    out=scores_ps[:, s * BPC : (s + 1) * BPC],
                    lhsT=aL[0:16, 384 + 128 * s : 384 + 128 * (s + 1)],
                    rhs=pre16[:],
                    start=False, stop=False, skip_group_check=True,
                )

            # ---- transpose gathered strips, score matmuls (fp32r) ---------
            qgT_sb = sp.tile([D, NS * 128], F32R)
            for s in range(NS):
                tr_ps = pp.tile([D, 128], F32, tag=f"tr{s % 2}")
                nc.tensor.transpose(tr_ps[:], qgB[:, s * D : (s + 1) * D].bitcast(F32), ident)
                nc.vector.tensor_copy(qgT_sb[:, s * 128 : (s + 1) * 128], tr_ps[:])
                nc.tensor.matmul(
                    out=scores_ps[:, s * BPC : (s + 1) * BPC],
                    lhsT=qgT_sb[:, s * 128 : (s + 1) * 128],
                    rhs=vT_sb[:],
                    start=False, stop=(s == NS - 1), skip_group_check=True,
                )

            # ---- exp straight out of PSUM ---------------------------------
            e_sb = sp.tile([128, NS * BPC], F32R)
            nc.scalar.activation(out=e_sb[:], in_=scores_ps[:], func=ACT.Exp)

            # ---- denominator: S32[8s+b'] = sum_p e[p, 8s+b'] --------------
            S32_ps = comb2[0 : NS * BPC, BPC : BPC + 2]
            nc.tensor.matmul(out=S32_ps, lhsT=e_sb[:], rhs=ones2[:], start=True, stop=True, skip_group_check=True)

            # ---- unnormalized weighted sum (fp32r) ------------------------
            out_ps = pp.tile([BPC, D], F32)
            for s in range(NS):
                nc.tensor.matmul(
                    out=out_ps[:],
                    lhsT=e_sb[:, s * BPC : (s + 1) * BPC],
                    rhs=qgB[:, s * D : (s + 1) * D],
                    start=(s == 0), stop=(s == NS - 1),
                )

            # ---- fold strip sums via a constant matmul, normalize, store --
            S32_sb = sp.tile([NS * BPC, 2], F32)
            nc.vector.tensor_copy(S32_sb[:], S32_ps)
            S8_ps = pp.tile([BPC, 2], F32)
            nc.tensor.matmul(out=S8_ps[:], lhsT=fold8, rhs=S32_sb[:], start=True, stop=True)
            sinv = sp.tile([BPC, 1], F32)
            nc.vector.reciprocal(sinv[:], S8_ps[:, 0:1])
            outf = sp.tile([BPC, D], F32)
            nc.vector.tensor_scalar(
                out=outf[:], in0=out_ps[:], scalar1=sinv[:, 0:1], scalar2=None,
                op0=AOP.mult,
            )
            nc.sync.dma_start(out=out[:], in_=outf[:])

    nc.compile()
    return nc


_CACHE = {}


def _prep_in_maps(q, c_t, W_a, W_p):
    offs = (np.arange(R) - (R // 2 - 1)).astype(np.float32)  # [-3..4]
    p = np.arange(128)
    b_of = p // 16
    i_of = (p % 16) // 2
    jg_of = p % 2

    WPT = W_p.T.astype(np.float32)          # [256, 2]
    WPbig = np.zeros((256, 16), np.float32)
    WPbig[:, 0:8] = WPT[:, 0:1]
    WPbig[:, 8:16] = WPT[:, 1:2]

    oi16m_np = offs[np.arange(16) % 8] - 8388608.0

    E12c_np = np.zeros((16, 128), np.float32)
    E12c_np[i_of, p] = float(W)     # 128 * rm1_r[i(p), :]
    E12c_np[11, :] = 1.0            # + p1 (rr row 11 = c_3 = p1)

    selmask_np = (np.arange(BPC)[None, :] == b_of[:, None]).astype(np.float32)
    jgboff_np = (b_of * BSTR + PADB - W - 4 + 4 * jg_of).astype(np.float32)

    auxS_np = np.zeros((128, AUXS_W), np.float32)
    auxS_np[:, 16:32] = WPbig[0:128]
    auxS_np[:, 32:48] = WPbig[128:256]
    auxS_np[0:16, 48] = oi16m_np
    auxS_np[0:16, 49:177] = E12c_np
    auxS_np[:, 177:185] = selmask_np
    auxS_np[:, 185] = jgboff_np

    # B_s[q, p] = d(q, i(p)) + d(q, 8 + j(p, s)),  j = 4*jg(p) + s
    auxL_np = np.zeros((128, AUXL_W), np.float32)
    auxL_np[:, 0:128] = W_a.astype(np.float32)[0:128]
    auxL_np[:, 128:256] = W_a.astype(np.float32)[128:256]
    auxL_np[:, 256:384] = np.eye(128, dtype=np.float32)
    for s in range(NS):
        Bs = np.zeros((16, 128), np.float32)
        Bs[i_of, p] += 1.0
        Bs[8 + 4 * jg_of + s, p] += 1.0
        auxL_np[0:16, 384 + 128 * s : 384 + 128 * (s + 1)] = Bs
    bsel_np = (b_of[None, :] == np.arange(8)[:, None]).astype(np.float32)
    auxL_np[0:8, 896:1024] = bsel_np
    constm8_np = np.full((8, NS * BPC), FAKEC, np.float32)
    for r in range(8):
        for s in range(NS):
            constm8_np[r, 8 * s + r] = -BIGC
    auxL_np[0:8, 1024:1056] = constm8_np
    fold8_np = np.zeros((32, 8), np.float32)
    fold8_np[np.arange(32), np.arange(32) % 8] = 1.0
    auxL_np[0:32, 1056:1064] = fold8_np

    in_maps = []
    for c in range(NCORES):
        qs = q[c * BPC : (c + 1) * BPC]  # [BPC, D, H, W]
        qhw_np = np.ascontiguousarray(qs.transpose(0, 2, 3, 1)).reshape(BPC, HW, D)
        qtab_np = np.zeros((BPC, BSTR, D), np.float32)
        qtab_np[:, PADB : PADB + HW, :] = qhw_np
        ctT_np = np.ascontiguousarray(c_t[c * BPC : (c + 1) * BPC].T)  # [CSZ, BPC]
        auxS_c = auxS_np.copy()
        auxS_c[:, 0:8] = ctT_np[0:128]
        auxS_c[:, 8:16] = ctT_np[128:256]
        in_maps.append({
            "qtab": qtab_np.reshape(BPC * BSTR, D),
            "auxS": auxS_c,
            "auxL": auxL_np,
        })
    return in_maps


def run(trace=False, tmpdir=None, **inputs):
    q = np.asarray(inputs["q"], dtype=np.float32)
    c_t = np.asarray(inputs["c_t"], dtype=np.float32)
    W_a = np.asarray(inputs["W_a"], dtype=np.float32)
    W_p = np.asarray(inputs["W_p"], dtype=np.float32)
    if "nc" not in _CACHE:
        _CACHE["nc"] = _build()
    in_maps = _prep_in_maps(q, c_t, W_a, W_p)
    res = _bu.run_bass_kernel_spmd(
        _CACHE["nc"], in_maps, core_ids=list(range(NCORES)), trace=trace,
        tmpdir=tmpdir,
    )
    outp = np.concatenate([r["out"] for r in res.results], axis=0)
    return outp, res


def kernel(**inputs):
    outp, _ = run(trace=False, **inputs)
    return outp


# revision 30
# speedup vs baseline: 1.0704x; 1.0071x over previous
"""LocalAttention2d Bass kernel for 8 Trainium2 NeuronCores.

Strategy: pure data parallel over batch (8 batches/core).  The module only
attends over an 8x8 window of data-dependent spatial positions per batch.
All valid window columns are literally p1+offs (clipping only produces
invalid, masked positions), so each (batch, window-row, col-half) is ONE
contiguous 4-row block of the spatial table.  The kernel computes 128 block
indices on-device and fetches all 512 needed rows with a SINGLE indirect
DMA of 128 x 2KB descriptors from a host-padded [B*(H*W+16), D] table (the
+-8 row pads absorb edge blocks; those positions are invalid and masked).

Layout: partition p = b*16 + i*2 + jg (batch, window-row, col-half); the
index math runs on tiny [16, 8] r/c tiles, is expanded to [128, 1] block
indices by one selector matmul, and the Gaussian shift + validity mask are
folded into the scores PSUM with per-batch-constant and banded matmuls
using the linear mask -1024 + 512*(vr+vc) (exact in f32 at this scale), so
exp() reads PSUM directly.  Softmax is unnormalized: the weighted sum and
the denominator are both matmuls against exp scores, normalized at the end
by a per-partition reciprocal.  Score and weighted-sum matmuls run in
fp32r (single PE pass).

Host-side work is limited to data-INdependent layout prep (transposes of
q / c_t / W_p, constant tables, zero padding); every data-dependent step
(p_t, rounding, window indices, shift, softmax, output) runs on-device.
"""

import sys

import numpy as np

try:
    import concourse.bass_utils as _bu
except ImportError:  # fresh grading dir: fall back to the repo checkout
    sys.path.insert(0, "/opt/trn_rl_repo")
    import concourse.bass_utils as _bu

import concourse.bacc as bacc
import concourse.bass as bass
import concourse.mybir as mybir
import concourse.tile as tile
from concourse.bass import IndirectOffsetOnAxis

B, D, H, W = 64, 128, 128, 128
CSZ = 256
R = 8                     # window rows == cols
NCORES = 8
BPC = B // NCORES         # batches per core
HW = H * W
PADB = 132                # zero rows before each batch (absorbs rr_r=0 blocks)
PADE = 8                  # zero rows after each batch
BSTR = HW + PADB + PADE   # padded batch stride (rows)
ROWS = H + 1              # 129, padded row count in the reference
NS = 4                    # strips (col within 4-row block)
F32 = mybir.dt.float32
F32R = mybir.dt.float32r
I32 = mybir.dt.int32

AOP = mybir.AluOpType
ACT = mybir.ActivationFunctionType
AXL = mybir.AxisListType

BIGC = 1024.0             # mask constants: exact cancellation at 2^10 scale
HALFC = 512.0
FAKEC = -2048.0

# auxS [128, 186] (critical-path constants, first DMA):
#   0:8 ct0 | 8:16 ct1 | 16:32 WPbig0 | 32:48 WPbig1 (r/c-duplicated W_p)
#   | 48:49 oi16m (parts 0:16, offs - 2^23) | 49:177 E12c (parts 0:16)
#   | 177:185 selmask | 185:186 jgboff
AUXS_W = 186
# auxL [128, 1064]: 0:128 wa0 | 128:256 wa1 | 256:384 ident
#   | 384:896 B_s x4 (parts 0:16) | 896:1024 bsel (parts 0:8)
#   | 1024:1056 constm8 (parts 0:8) | 1056:1064 fold8 (parts 0:32)
AUXL_W = 1064


def _build():
    nc = bacc.Bacc(
        "TRN2",
        target_bir_lowering=False,
        debug=False,
        num_devices=NCORES,
    )

    qtab = nc.dram_tensor("qtab", [BPC * BSTR, D], F32, kind="ExternalInput")
    auxS = nc.dram_tensor("auxS", [128, AUXS_W], F32, kind="ExternalInput")
    auxL = nc.dram_tensor("auxL", [128, AUXL_W], F32, kind="ExternalInput")
    out = nc.dram_tensor("out", [BPC, D], F32, kind="ExternalOutput")

    with tile.TileContext(nc) as tc:
        with (
            tc.tile_pool(name="sb", bufs=1) as sp,
            tc.tile_pool(name="ps", bufs=1, space="PSUM") as pp,
        ):
            # ---- input DMAs: small critical constants first ---------------
            aS = sp.tile([128, AUXS_W], F32)
            nc.sync.dma_start(out=aS[:], in_=auxS[:])
            aL = sp.tile([128, AUXL_W], F32)
            nc.sync.dma_start(out=aL[:], in_=auxL[:])

            ct0 = aS[:, 0:8]
            ct1 = aS[:, 8:16]
            wpb0 = aS[:, 16:32]
            wpb1 = aS[:, 32:48]
            oi16m = aS[0:16, 48:49]
            E12c = aS[0:16, 49:177]
            selmask = aS[:, 177:185]
            jgboff = aS[:, 185:186]
            wa0 = aL[:, 0:128]
            wa1 = aL[:, 128:256]
            ident = aL[:, 256:384]
            bsel = aL[0:8, 896:1024]
            constm8 = aL[0:8, 1024:1056]
            fold8 = aL[0:32, 1056:1064]

            ones2 = sp.tile([128, 2], F32R)
            nc.gpsimd.memset(ones2[:].bitcast(F32), 1.0)

            # ---- p_t logits straight to [16,8] via duplicated W_p weights;
            # rows 0:8 = r-part (p_t[:,0]), rows 8:16 = c-part (p_t[:,1]) ----
            ptcomb = pp.tile([16, BPC], F32)
            nc.tensor.matmul(out=ptcomb[:], lhsT=wpb0, rhs=ct0, start=True, stop=False)
            nc.tensor.matmul(out=ptcomb[:], lhsT=wpb1, rhs=ct1, start=False, stop=True)
            # sigmoid fully on the vector engine (the scalar engine's
            # completion semaphore takes ~3us to become visible, so any
            # scalar activation on the critical path stalls the chain).
            # WPbig carries -log2(e) so the PSUM already holds y = -x*log2e;
            # sigmoid(x) = 1/(1 + 2^y) with 2^y = 2^round(y) * 2^frac:
            # exponent-field build in f32 (exact: (n+127)*2^23 needs 8
            # mantissa bits), quartic minimax for 2^frac on [-.5, .5]
            nr = sp.tile([16, BPC], F32)
            nc.vector.tensor_scalar(
                out=nr[:], in0=ptcomb[:], scalar1=12582912.0, scalar2=12582912.0,
                op0=AOP.add, op1=AOP.subtract,
            )
            fr = sp.tile([16, BPC], F32)
            nc.vector.tensor_tensor(out=fr[:], in0=ptcomb[:], in1=nr[:], op=AOP.subtract)
            bitsf = sp.tile([16, BPC], F32)
            nc.vector.tensor_scalar(
                out=bitsf[:], in0=nr[:], scalar1=127.0, scalar2=8388608.0,
                op0=AOP.add, op1=AOP.mult,
            )
            bitsi = sp.tile([16, BPC], I32)
            nc.vector.tensor_copy(bitsi[:], bitsf[:])
            e1 = sp.tile([16, BPC], F32)
            nc.vector.tensor_scalar(
                out=e1[:], in0=fr[:], scalar1=0.6931273, scalar2=1.0,
                op0=AOP.mult, op1=AOP.add,
            )
            e2 = sp.tile([16, BPC], F32)
            nc.vector.tensor_scalar(
                out=e2[:], in0=fr[:], scalar1=0.055875517, scalar2=0.24022211,
                op0=AOP.mult, op1=AOP.add,
            )
            f2 = sp.tile([16, BPC], F32)
            nc.vector.tensor_tensor(out=f2[:], in0=fr[:], in1=fr[:], op=AOP.mult)
            e2b = sp.tile([16, BPC], F32)
            nc.vector.scalar_tensor_tensor(
                out=e2b[:], in0=f2[:], scalar=0.009670765, in1=e2[:],
                op0=AOP.mult, op1=AOP.add,
            )
            tq = sp.tile([16, BPC], F32)
            nc.vector.tensor_tensor(out=tq[:], in0=f2[:], in1=e2b[:], op=AOP.mult)
            pq = sp.tile([16, BPC], F32)
            nc.vector.tensor_tensor(out=pq[:], in0=tq[:], in1=e1[:], op=AOP.add)
            p2y = sp.tile([16, BPC], F32)
            nc.vector.tensor_tensor(
                out=p2y[:], in0=pq[:], in1=bitsi[:].bitcast(F32), op=AOP.mult
            )
            one16 = sp.tile([16, BPC], F32)
            nc.vector.tensor_scalar(
                out=one16[:], in0=p2y[:], scalar1=1.0, scalar2=None, op0=AOP.add
            )
            sig16 = sp.tile([16, BPC], F32)
            nc.vector.reciprocal(sig16[:], one16[:])

            # ---- 128*sig + 2^23 rounds to integer; the -2^23 is folded
            # into oi16m = offs - 2^23 -----------------------------------
            prfraw = sp.tile([16, BPC], F32)
            nc.vector.tensor_scalar(
                out=prfraw[:], in0=sig16[:], scalar1=128.0, scalar2=8388608.0,
                op0=AOP.mult, op1=AOP.add,
            )

            # ---- window values: a=max(p+o,0); rr=a*(a<129); rm1=max(rr-1,0)
            aa = sp.tile([16, BPC], F32)
            nc.vector.tensor_scalar(
                out=aa[:], in0=prfraw[:], scalar1=oi16m, scalar2=0.0,
                op0=AOP.add, op1=AOP.max,
            )
            amask = sp.tile([16, BPC], F32)
            nc.vector.tensor_scalar(
                out=amask[:], in0=aa[:], scalar1=float(ROWS), scalar2=None, op0=AOP.is_lt
            )
            rr = sp.tile([16, BPC], F32)
            nc.vector.tensor_tensor(out=rr[:], in0=aa[:], in1=amask[:], op=AOP.mult)
            # ---- block indices: one selector matmul + diag pick -----------
            # idx8[p, b'] = 128*(rr_r[i(p), b'] - 1) + p1[b']; the -128 is
            # folded into jgboff and invalid rows (rr_r=0) land in the front
            # pad (row 11 of rr = c_3 = clip-mod(p1) = p1 exactly)
            comb2 = pp.tile([128, BPC + 2], F32)
            idx8_ps = comb2[:, 0:BPC]
            nc.tensor.matmul(out=idx8_ps, lhsT=E12c, rhs=rr[:], start=True, stop=True, skip_group_check=True)
            m1 = sp.tile([128, BPC], F32)
            nc.vector.tensor_tensor(out=m1[:], in0=idx8_ps, in1=selmask, op=AOP.mult)
            red = sp.tile([128, 1], F32)
            nc.vector.tensor_reduce(out=red[:], in_=m1[:], axis=AXL.X, op=AOP.add)
            idxs = sp.tile([128, 1], F32)
            nc.vector.tensor_scalar(
                out=idxs[:], in0=red[:], scalar1=jgboff, scalar2=None, op0=AOP.add
            )
            idx128 = sp.tile([128, 1], I32)
            nc.vector.tensor_copy(idx128[:], idxs[:])

            # ---- THE gather: one DMA, 128 x 2KB blocks --------------------
            qgB = sp.tile([128, NS * D], F32R)
            nc.gpsimd.indirect_dma_start(
                out=qgB[:],
                out_offset=None,
                in_=qtab[:].bitcast(F32R),
                in_offset=IndirectOffsetOnAxis(ap=idx128[:, 0:1], axis=0),
            )

            # ---- shift/valid pre-term (overlaps the gather) ---------------
            # pre16 = 512*(rr>0) - (rm1 - p_t)^2/8 per r/c component
            rpos = sp.tile([16, BPC], F32)
            nc.vector.tensor_scalar(
                out=rpos[:], in0=rr[:], scalar1=0.0, scalar2=None, op0=AOP.is_gt
            )
            rm1f = sp.tile([16, BPC], F32)
            nc.vector.tensor_scalar(
                out=rm1f[:], in0=rr[:], scalar1=1.0, scalar2=0.0,
                op0=AOP.subtract, op1=AOP.max,
            )
            rexpd = sp.tile([16, BPC], F32)
            nc.vector.scalar_tensor_tensor(
                out=rexpd[:], in0=sig16[:], scalar=-128.0, in1=rm1f[:],
                op0=AOP.mult, op1=AOP.add,
            )
            sq = sp.tile([16, BPC], F32)
            nc.vector.tensor_tensor(out=sq[:], in0=rexpd[:], in1=rexpd[:], op=AOP.mult)
            tsA = sp.tile([16, BPC], F32)
            nc.vector.tensor_scalar(
                out=tsA[:], in0=sq[:], scalar1=-0.125, scalar2=None, op0=AOP.mult
            )
            pre16 = sp.tile([16, BPC], F32)
            nc.vector.scalar_tensor_tensor(
                out=pre16[:], in0=rpos[:], scalar=HALFC, in1=tsA[:],
                op0=AOP.mult, op1=AOP.add,
            )

            # ---- vT[d,b] = sum_c W_a[c,d] c_t[c,b] ------------------------
            vT_ps = pp.tile([D, BPC], F32)
            nc.tensor.matmul(out=vT_ps[:], lhsT=wa0, rhs=ct0, start=True, stop=False)
            nc.tensor.matmul(out=vT_ps[:], lhsT=wa1, rhs=ct1, start=False, stop=True)
            vT_sb = sp.tile([D, BPC], F32R)
            nc.vector.tensor_copy(vT_sb[:], vT_ps[:])

            # ---- scores PSUM: per-batch consts + banded shift/mask --------
            # (1024-scale constants: order-independent, no cancellation loss)
            scores_ps = pp.tile([128, NS * BPC], F32)
            nc.tensor.matmul(
                out=scores_ps[:], lhsT=bsel, rhs=constm8,
                start=True, stop=False, skip_group_check=True,
            )
            for s in range(NS):
                nc.tensor.matmul(
                    out=scores_ps[:, s * BPC : (s + 1) * BPC],
                    lhsT=aL[0:16, 384 + 128 * s : 384 + 128 * (s + 1)],
                    rhs=pre16[:],
                    start=False, stop=False, skip_group_check=True,
                )

            # ---- transpose gathered strips, score matmuls (fp32r) ---------
            qgT_sb = sp.tile([D, NS * 128], F32R)
            for s in range(NS):
                tr_ps = pp.tile([D, 128], F32, tag=f"tr{s % 2}")
                nc.tensor.transpose(tr_ps[:], qgB[:, s * D : (s + 1) * D].bitcast(F32), ident)
                nc.vector.tensor_copy(qgT_sb[:, s * 128 : (s + 1) * 128], tr_ps[:])
                nc.tensor.matmul(
                    out=scores_ps[:, s * BPC : (s + 1) * BPC],
                    lhsT=qgT_sb[:, s * 128 : (s + 1) * 128],
                    rhs=vT_sb[:],
                    start=False, stop=(s == NS - 1), skip_group_check=True,
                )

            # ---- exp straight out of PSUM ---------------------------------
            e_sb = sp.tile([128, NS * BPC], F32R)
            nc.scalar.activation(out=e_sb[:], in_=scores_ps[:], func=ACT.Exp)

            # ---- denominator: S32[8s+b'] = sum_p e[p, 8s+b'] --------------
            S32_ps = comb2[0 : NS * BPC, BPC : BPC + 2]
            nc.tensor.matmul(out=S32_ps, lhsT=e_sb[:], rhs=ones2[:], start=True, stop=True, skip_group_check=True)

            # ---- unnormalized weighted sum (fp32r) ------------------------
            out_ps = pp.tile([BPC, D], F32)
            for s in range(NS):
                nc.tensor.matmul(
                    out=out_ps[:],
                    lhsT=e_sb[:, s * BPC : (s + 1) * BPC],
                    rhs=qgB[:, s * D : (s + 1) * D],
                    start=(s == 0), stop=(s == NS - 1),
                )

            # ---- fold strip sums via a constant matmul, normalize, store --
            S32_sb = sp.tile([NS * BPC, 2], F32)
            nc.vector.tensor_copy(S32_sb[:], S32_ps)
            S8_ps = pp.tile([BPC, 2], F32)
            nc.tensor.matmul(out=S8_ps[:], lhsT=fold8, rhs=S32_sb[:], start=True, stop=True)
            sinv = sp.tile([BPC, 1], F32)
            nc.vector.reciprocal(sinv[:], S8_ps[:, 0:1])
            outf = sp.tile([BPC, D], F32)
            nc.vector.tensor_scalar(
                out=outf[:], in0=out_ps[:], scalar1=sinv[:, 0:1], scalar2=None,
                op0=AOP.mult,
            )
            nc.sync.dma_start(out=out[:], in_=outf[:])

    nc.compile()
    return nc


_CACHE = {}


def _prep_in_maps(q, c_t, W_a, W_p):
    offs = (np.arange(R) - (R // 2 - 1)).astype(np.float32)  # [-3..4]
    p = np.arange(128)
    b_of = p // 16
    i_of = (p % 16) // 2
    jg_of = p % 2

    WPT = (W_p.T * -1.4426950408889634).astype(np.float32)   # [256, 2] * -log2e
    WPbig = np.zeros((256, 16), np.float32)
    WPbig[:, 0:8] = WPT[:, 0:1]
    WPbig[:, 8:16] = WPT[:, 1:2]

    oi16m_np = offs[np.arange(16) % 8] - 8388608.0

    E12c_np = np.zeros((16, 128), np.float32)
    E12c_np[i_of, p] = float(W)     # 128 * rm1_r[i(p), :]
    E12c_np[11, :] = 1.0            # + p1 (rr row 11 = c_3 = p1)

    selmask_np = (np.arange(BPC)[None, :] == b_of[:, None]).astype(np.float32)
    jgboff_np = (b_of * BSTR + PADB - W - 4 + 4 * jg_of).astype(np.float32)

    auxS_np = np.zeros((128, AUXS_W), np.float32)
    auxS_np[:, 16:32] = WPbig[0:128]
    auxS_np[:, 32:48] = WPbig[128:256]
    auxS_np[0:16, 48] = oi16m_np
    auxS_np[0:16, 49:177] = E12c_np
    auxS_np[:, 177:185] = selmask_np
    auxS_np[:, 185] = jgboff_np

    # B_s[q, p] = d(q, i(p)) + d(q, 8 + j(p, s)),  j = 4*jg(p) + s
    auxL_np = np.zeros((128, AUXL_W), np.float32)
    auxL_np[:, 0:128] = W_a.astype(np.float32)[0:128]
    auxL_np[:, 128:256] = W_a.astype(np.float32)[128:256]
    auxL_np[:, 256:384] = np.eye(128, dtype=np.float32)
    for s in range(NS):
        Bs = np.zeros((16, 128), np.float32)
        Bs[i_of, p] += 1.0
        Bs[8 + 4 * jg_of + s, p] += 1.0
        auxL_np[0:16, 384 + 128 * s : 384 + 128 * (s + 1)] = Bs
    bsel_np = (b_of[None, :] == np.arange(8)[:, None]).astype(np.float32)
    auxL_np[0:8, 896:1024] = bsel_np
    constm8_np = np.full((8, NS * BPC), FAKEC, np.float32)
    for r in range(8):
        for s in range(NS):
            constm8_np[r, 8 * s + r] = -BIGC
    auxL_np[0:8, 1024:1056] = constm8_np
    fold8_np = np.zeros((32, 8), np.float32)
    fold8_np[np.arange(32), np.arange(32) % 8] = 1.0
    auxL_np[0:32, 1056:1064] = fold8_np

    in_maps = []
    for c in range(NCORES):
        qs = q[c * BPC : (c + 1) * BPC]  # [BPC, D, H, W]
        qhw_np = np.ascontiguousarray(qs.transpose(0, 2, 3, 1)).reshape(BPC, HW, D)
        qtab_np = np.zeros((BPC, BSTR, D), np.float32)
        qtab_np[:, PADB : PADB + HW, :] = qhw_np
        ctT_np = np.ascontiguousarray(c_t[c * BPC : (c + 1) * BPC].T)  # [CSZ, BPC]
        auxS_c = auxS_np.copy()
        auxS_c[:, 0:8] = ctT_np[0:128]
        auxS_c[:, 8:16] = ctT_np[128:256]
        in_maps.append({
            "qtab": qtab_np.reshape(BPC * BSTR, D),
            "auxS": auxS_c,
            "auxL": auxL_np,
        })
    return in_maps


def run(trace=False, tmpdir=None, **inputs):
    q = np.asarray(inputs["q"], dtype=np.float32)
    c_t = np.asarray(inputs["c_t"], dtype=np.float32)
    W_a = np.asarray(inputs["W_a"], dtype=np.float32)
    W_p = np.asarray(inputs["W_p"], dtype=np.float32)
    if "nc" not in _CACHE:
        _CACHE["nc"] = _build()
    in_maps = _prep_in_maps(q, c_t, W_a, W_p)
    res = _bu.run_bass_kernel_spmd(
        _CACHE["nc"], in_maps, core_ids=list(range(NCORES)), trace=trace,
        tmpdir=tmpdir,
    )
    outp = np.concatenate([r["out"] for r in res.results], axis=0)
    return outp, res


def kernel(**inputs):
    outp, _ = run(trace=False, **inputs)
    return outp
